# revision 6
# baseline (speedup 1.0000x reference)
"""Trainium2 Bass kernel for nn_Attention_31104153158132.

Edge-bias multi-head attention block (GNN message passing):
  - QKV projections + per-head scores
  - Edge-coefficient MLP over all S*S pairs (dominant cost: ~17.2 GFLOP/core)
  - softmax(alpha*scores + beta*edge_bias) attention
  - output projection + LN + GELU FFN + LN

Sharding: data-parallel over batch. BS=8 batches -> one batch element per
NeuronCore (8 cores). Weights replicated. No collectives.

Layout strategy per core:
  - Edge MLP runs as a transposed-activation chain (features on partitions,
    edge-pair rows on the free dim) so the 512x512 weight matrices act as the
    stationary matmul operand and activations stream with N=512.
  - All big matmuls use float32r (full PE speed at N>=256, ~1e-4 accuracy).
  - edge_attr row-tiles are transposed on the PE (identity matmul).
"""

import numpy as np

import concourse.bass as bass
import concourse.tile as tile
from concourse import bacc, mybir
from concourse.masks import make_identity

BS, S, H, NH = 8, 128, 512, 8
DH = H // NH          # 64
EP = S * S            # 16384 edge pairs per batch
RT = 512              # edge rows per tile
NRT = EP // RT        # 32 row tiles
KC = H // 128         # 4 contraction chunks of 128

F32 = mybir.dt.float32
F32R = mybir.dt.float32r
AF = mybir.ActivationFunctionType
ALU = mybir.AluOpType
AX = mybir.AxisListType

LN_EPS = 1e-5



def _bcast(ap, p=128):
    """Prepend a stride-0 partition dim: [n] -> [p, n] broadcast DMA source."""
    return bass.AP(tensor=ap.tensor, offset=ap.offset, ap=[[0, p]] + list(ap.ap))

def _load_w_r(nc, wstage, dst_pool, dram_ap, kparts, n, name):
    """DMA a [K, N] weight into [128, K//128, N] f32 staging, then round to a
    float32r tile (fp32r matmul operands must be produced by a compute op)."""
    stg = wstage.tile([128, KC, 512], F32, tag="wstage")
    stg_v = stg[:, : kparts, : n]
    nc.sync.dma_start(stg_v, dram_ap.rearrange("(k p) n -> p k n", p=128))
    w_r = dst_pool.tile([128, kparts, n], F32R, tag=name)
    nc.vector.tensor_copy(w_r[:], stg_v)
    return w_r


def build(gelu_af=None):
    gelu_af = gelu_af or AF.Gelu
    nc = bacc.Bacc("TRN2", target_bir_lowering=False, debug=False, num_devices=8)

    t_in = lambda name, shape: nc.dram_tensor(name, shape, F32, kind="ExternalInput").ap()
    x = t_in("x", [S, H])
    edge_attr = t_in("edge_attr", [EP, H])
    Wq, bq = t_in("Wq", [H, H]), t_in("bq", [H])
    Wk, bk = t_in("Wk", [H, H]), t_in("bk", [H])
    Wv, bv = t_in("Wv", [H, H]), t_in("bv", [H])
    Wo, bo = t_in("Wo", [H, H]), t_in("bo", [H])
    We1, be1 = t_in("We1", [H, H]), t_in("be1", [H])
    We2, be2 = t_in("We2", [H, H]), t_in("be2", [H])
    We3, be3 = t_in("We3", [H, NH]), t_in("be3", [NH])
    Wp1, bp1 = t_in("Wp1", [H, 2 * H]), t_in("bp1", [2 * H])
    Wp2, bp2 = t_in("Wp2", [2 * H, H]), t_in("bp2", [H])
    sa_g, sa_b = t_in("sa_g", [H]), t_in("sa_b", [H])
    on_g, on_b = t_in("on_g", [H]), t_in("on_b", [H])
    alpha, beta = t_in("alpha", [1]), t_in("beta", [1])
    out = nc.dram_tensor("out", [S, H], F32, kind="ExternalOutput").ap()

    with tile.TileContext(nc) as tc:
        with (
            tc.tile_pool(name="consts", bufs=1) as consts,
            tc.tile_pool(name="persist", bufs=1) as persist,
            tc.tile_pool(name="wstage", bufs=2) as wstage,
        ):
            ident = consts.tile([128, 128], F32)
            make_identity(nc, ident[:])

            # live across phases
            xpb = persist.tile([128, H], F32)            # x + bo (residual + out-proj bias)
            v_r = persist.tile([128, H], F32R)           # v, rows on partitions
            scores_sb = persist.tile([128, NH, S], F32)  # q@kT per head
            ssum_all = persist.tile([128, NH], F32)      # softmax row sums
            bias_all = persist.tile([128, NH, S], F32)   # edge bias, [q, h, k]

            cf_ctx = tc.tile_pool(name="coeffs", bufs=1)
            cfp = cf_ctx.__enter__()
            coeffsT = cfp.tile([NH, EP], F32)            # edge-MLP output, transposed

            # ---------------- Phase A: QKV + scores ----------------
            with (
                tc.tile_pool(name="pa_sb", bufs=1) as pa,
                tc.tile_pool(name="pa_w", bufs=1) as paw,
                tc.tile_pool(name="pa_ps", bufs=2, space="PSUM") as paps,
                tc.tile_pool(name="pa_tp", bufs=2, space="PSUM") as patp,
            ):
                x_sb = pa.tile([128, H], F32)
                nc.sync.dma_start(x_sb[:], x[:, :])
                bo_big = pa.tile([128, H], F32)
                nc.sync.dma_start(bo_big[:], _bcast(bo))
                nc.vector.tensor_add(xpb[:], x_sb[:], bo_big[:])

                xT_r = pa.tile([128, KC, 128], F32R)
                for j in range(KC):
                    pt = patp.tile([128, 128], F32, tag="pa_tp")
                    nc.tensor.transpose(pt[:], x_sb[:, j * 128:(j + 1) * 128], ident[:])
                    nc.vector.tensor_copy(xT_r[:, j, :], pt[:])

                qkv_sb = {}
                for nm, W, b in (("q", Wq, bq), ("k", Wk, bk), ("v", Wv, bv)):
                    w_r = _load_w_r(nc, wstage, paw, W, KC, 512, f"w{nm}")
                    b_big = pa.tile([128, H], F32, tag=f"bbig{nm}")
                    nc.sync.dma_start(b_big[:], _bcast(b))
                    ps = paps.tile([128, 512], F32, tag="pa_ps")
                    for k in range(KC):
                        nc.tensor.matmul(ps[:], xT_r[:, k, :], w_r[:, k, :],
                                         start=(k == 0), stop=(k == KC - 1))
                    if nm == "v":
                        nc.vector.tensor_add(v_r[:], ps[:], b_big[:])
                    else:
                        t_sb = pa.tile([128, H], F32, tag=f"{nm}sb")
                        nc.vector.tensor_add(t_sb[:], ps[:], b_big[:])
                        qkv_sb[nm] = t_sb

                # per-head transposed q/k: [64(dh), NH, 128(row)]
                qT_r = pa.tile([64, NH, 128], F32R, tag="qT")
                kT_r = pa.tile([64, NH, 128], F32R, tag="kT")
                for nm, dst in (("q", qT_r), ("k", kT_r)):
                    src = qkv_sb[nm]
                    for h in range(NH):
                        pt = patp.tile([128, 128], F32, tag="pa_tp")
                        nc.tensor.transpose(pt[:64, :], src[:, h * DH:(h + 1) * DH], ident[:])
                        nc.vector.tensor_copy(dst[:, h, :], pt[:64, :])

                for h in range(NH):
                    ps = patp.tile([128, 128], F32, tag="pa_sc")
                    nc.tensor.matmul(ps[:], qT_r[:, h, :], kT_r[:, h, :])
                    nc.vector.tensor_copy(scores_sb[:, h, :], ps[:])

            # ---------------- Phase B: edge-coefficient MLP ----------------
            with (
                tc.tile_pool(name="eb_w", bufs=1) as ebw,
                tc.tile_pool(name="eb_sb", bufs=2) as eb,
                tc.tile_pool(name="eb_tp", bufs=2, space="PSUM") as ebtp,
                tc.tile_pool(name="eb_mm", bufs=2, space="PSUM") as ebmm,
                tc.tile_pool(name="eb_l3", bufs=2, space="PSUM") as ebl3,
            ):
                we1_r = _load_w_r(nc, wstage, ebw, We1, KC, 512, "we1")
                we2_r = _load_w_r(nc, wstage, ebw, We2, KC, 512, "we2")
                we3_stg = wstage.tile([128, KC, 512], F32, tag="wstage")
                nc.sync.dma_start(we3_stg[:, :, :NH],
                                  We3.rearrange("(k p) n -> p k n", p=128))
                we3_r = ebw.tile([128, KC, NH], F32R)
                nc.vector.tensor_copy(we3_r[:], we3_stg[:, :, :NH])

                be1_col = ebw.tile([128, KC], F32)
                nc.sync.dma_start(be1_col[:], be1.rearrange("(k p) -> p k", p=128))
                be2_col = ebw.tile([128, KC], F32)
                nc.sync.dma_start(be2_col[:], be2.rearrange("(k p) -> p k", p=128))
                be3_col = ebw.tile([NH, 1], F32)
                nc.sync.dma_start(be3_col[:], be3.rearrange("(o p) -> p o", o=1))

                for t in range(NRT):
                    r0 = t * RT
                    eraw = eb.tile([128, KC, 512], F32, tag="eraw")
                    nc.sync.dma_start(
                        eraw[:],
                        edge_attr[r0:r0 + RT, :].rearrange("(i p) h -> p i h", p=128))

                    et = eb.tile([128, KC, 512], F32R, tag="et")
                    for j in range(KC):
                        for i in range(KC):
                            pt = ebtp.tile([128, 128], F32, tag="eb_tp")
                            nc.tensor.transpose(
                                pt[:], eraw[:, i, j * 128:(j + 1) * 128], ident[:])
                            nc.vector.tensor_copy(et[:, j, i * 128:(i + 1) * 128], pt[:])

                    h1t = eb.tile([128, KC, 512], F32R, tag="h1t")
                    for c in range(KC):
                        ps = ebmm.tile([128, 512], F32, tag="eb_mm")
                        for k in range(KC):
                            nc.tensor.matmul(
                                ps[:], we1_r[:, k, c * 128:(c + 1) * 128], et[:, k, :],
                                start=(k == 0), stop=(k == KC - 1))
                        nc.vector.tensor_scalar(
                            h1t[:, c, :], ps[:], be1_col[:, c:c + 1], 0.0,
                            op0=ALU.add, op1=ALU.max)

                    h2t = eb.tile([128, KC, 512], F32R, tag="h2t")
                    for c in range(KC):
                        ps = ebmm.tile([128, 512], F32, tag="eb_mm")
                        for k in range(KC):
                            nc.tensor.matmul(
                                ps[:], we2_r[:, k, c * 128:(c + 1) * 128], h1t[:, k, :],
                                start=(k == 0), stop=(k == KC - 1))
                        nc.vector.tensor_scalar(
                            h2t[:, c, :], ps[:], be2_col[:, c:c + 1], 0.0,
                            op0=ALU.add, op1=ALU.max)

                    ps3 = ebl3.tile([NH, 512], F32, tag="eb_l3")
                    for k in range(KC):
                        nc.tensor.matmul(ps3[:], we3_r[:, k, :], h2t[:, k, :],
                                         start=(k == 0), stop=(k == KC - 1))
                    nc.vector.tensor_scalar_add(
                        coeffsT[:, r0:r0 + RT], ps3[:], be3_col[:, 0:1])

            # gather edge bias into per-head [q, k] layout, then free coeffsT
            for h in range(NH):
                nc.sync.dma_start(
                    bias_all[:, h, :],
                    coeffsT[h:h + 1, :].rearrange("o (q k) -> o q k", k=S))
            cf_ctx.__exit__(None, None, None)

            # ---------------- Phase C: softmax attention + FFN ----------------
            with (
                tc.tile_pool(name="pc_sb", bufs=2) as pc,
                tc.tile_pool(name="pc_w", bufs=1) as pcw,
                tc.tile_pool(name="pc_tp", bufs=2, space="PSUM") as pctp,
                tc.tile_pool(name="pc_at", bufs=2, space="PSUM") as pcat,
                tc.tile_pool(name="pc_mm", bufs=2, space="PSUM") as pcmm,
            ):
                # weights / constants for this phase.
                # Wo staged head-major: wo_stg[p, h, n] = Wo[h*64+p, n] so both
                # matmul operands of the output projection sit at partition 0.
                wo_stg = pcw.tile([64, NH, 512], F32, tag="wostage")
                nc.sync.dma_start(wo_stg[:], Wo.rearrange("(h p) n -> p h n", p=DH))
                wo_hr = pcw.tile([64, NH, 512], F32R)
                nc.vector.tensor_copy(wo_hr[:], wo_stg[:])

                wp1a_r = _load_w_r(nc, wstage, pcw, Wp1[:, 0:512], KC, 512, "wp1a")
                wp1b_r = _load_w_r(nc, wstage, pcw, Wp1[:, 512:1024], KC, 512, "wp1b")
                wp2a_r = _load_w_r(nc, wstage, pcw, Wp2[0:512, :], KC, 512, "wp2a")
                wp2b_r = _load_w_r(nc, wstage, pcw, Wp2[512:1024, :], KC, 512, "wp2b")

                def bbig(b_ap, n, nm):
                    t = pcw.tile([128, n], F32, tag=nm)
                    nc.sync.dma_start(t[:], _bcast(b_ap))
                    return t

                bp1_big = bbig(bp1, 2 * H, "bp1")
                bp2_big = bbig(bp2, H, "bp2")
                sag_big = bbig(sa_g, H, "sag")
                sab_big = bbig(sa_b, H, "sab")
                ong_big = bbig(on_g, H, "ong")
                onb_big = bbig(on_b, H, "onb")

                al_col = pcw.tile([128, 1], F32)
                nc.sync.dma_start(al_col[:], _bcast(alpha))
                al8_col = pcw.tile([128, 1], F32)
                # fold the 1/sqrt(DH) q-scaling into alpha
                nc.vector.tensor_scalar_mul(al8_col[:], al_col[:], 1.0 / 8.0)
                be_col = pcw.tile([128, 1], F32)
                nc.sync.dma_start(be_col[:], _bcast(beta))
                eps_col = pcw.tile([128, 1], F32)
                nc.vector.memset(eps_col[:], LN_EPS)

                # softmax + attention per head
                attnT_sb = pcw.tile([64, NH, 128], F32R)
                for h in range(NH):
                    bb = pc.tile([128, S], F32, tag="bb")
                    nc.vector.tensor_scalar_mul(bb[:], bias_all[:, h, :], be_col[:, 0:1])
                    z = pc.tile([128, S], F32, tag="z")
                    nc.vector.scalar_tensor_tensor(
                        z[:], scores_sb[:, h, :], al8_col[:, 0:1], bb[:],
                        op0=ALU.mult, op1=ALU.add)
                    nm_t = pc.tile([128, 1], F32, tag="nm")
                    nc.vector.reduce_max(nm_t[:], z[:], axis=AX.X, negate=True)
                    e_t = pc.tile([128, S], F32, tag="e")
                    nc.scalar.activation(e_t[:], z[:], AF.Exp, bias=nm_t[:, 0:1],
                                         scale=1.0, accum_out=ssum_all[:, h:h + 1])
                    r_t = pc.tile([128, 1], F32, tag="r")
                    nc.vector.reciprocal(r_t[:], ssum_all[:, h:h + 1])
                    en_t = pc.tile([128, S], F32, tag="en")
                    nc.vector.tensor_scalar_mul(en_t[:], e_t[:], r_t[:, 0:1])
                    pt = pctp.tile([128, 128], F32, tag="pc_tp")
                    nc.tensor.transpose(pt[:], en_t[:], ident[:])
                    eT_r = pc.tile([128, S], F32R, tag="eT")
                    nc.vector.tensor_copy(eT_r[:], pt[:])
                    aps = pcat.tile([64, 128], F32, tag="pc_at")
                    nc.tensor.matmul(aps[:], v_r[:, h * DH:(h + 1) * DH], eT_r[:])
                    nc.vector.tensor_copy(attnT_sb[:, h, :], aps[:])

                # output projection: out1 = attn @ Wo  (+ bo + x via xpb)
                ps_o = pcmm.tile([128, 512], F32, tag="pc_mm")
                for h in range(NH):
                    nc.tensor.matmul(ps_o[:], attnT_sb[:, h, :], wo_hr[:, h, :],
                                     start=(h == 0), stop=(h == NH - 1))
                t1 = pc.tile([128, H], F32, tag="t1")
                nc.vector.tensor_add(t1[:], ps_o[:], xpb[:])

                def layernorm(dst, src, g_big, b_big, tag):
                    st = pc.tile([128, 6], F32, tag=f"{tag}_st")
                    nc.vector.bn_stats(st[:], src[:])
                    mv = pc.tile([128, 2], F32, tag=f"{tag}_mv")
                    nc.vector.bn_aggr(mv[:], st[:])
                    sv = pc.tile([128, 1], F32, tag=f"{tag}_sv")
                    nc.scalar.activation(sv[:], mv[:, 1:2], AF.Sqrt,
                                         bias=eps_col[:, 0:1], scale=1.0)
                    rstd = pc.tile([128, 1], F32, tag=f"{tag}_rs")
                    nc.vector.reciprocal(rstd[:], sv[:])
                    tmp = pc.tile([128, H], F32, tag=f"{tag}_tmp")
                    nc.vector.scalar_tensor_tensor(
                        tmp[:], src[:], mv[:, 0:1], g_big[:],
                        op0=ALU.subtract, op1=ALU.mult)
                    nc.vector.scalar_tensor_tensor(
                        dst[:], tmp[:], rstd[:, 0:1], b_big[:],
                        op0=ALU.mult, op1=ALU.add)

                res = pc.tile([128, H], F32, tag="res")
                layernorm(res, t1, sag_big, sab_big, "ln1")

                # FFN: gelu(res @ Wp1 + bp1) @ Wp2 + bp2
                resT_r = pc.tile([128, KC, 128], F32R, tag="resT")
                for j in range(KC):
                    pt = pctp.tile([128, 128], F32, tag="pc_tp")
                    nc.tensor.transpose(pt[:], res[:, j * 128:(j + 1) * 128], ident[:])
                    nc.vector.tensor_copy(resT_r[:, j, :], pt[:])

                g_sb = pc.tile([128, 2 * H], F32, tag="gsb")
                for half, w_r in ((0, wp1a_r), (1, wp1b_r)):
                    ps = pcmm.tile([128, 512], F32, tag="pc_mm")
                    for k in range(KC):
                        nc.tensor.matmul(ps[:], resT_r[:, k, :], w_r[:, k, :],
                                         start=(k == 0), stop=(k == KC - 1))
                    tg = pc.tile([128, 512], F32, tag="tg")
                    nc.vector.tensor_add(tg[:], ps[:], bp1_big[:, half * 512:(half + 1) * 512])
                    nc.scalar.activation(g_sb[:, half * 512:(half + 1) * 512], tg[:], gelu_af)

                gT_r = pc.tile([128, 2 * KC, 128], F32R, tag="gT")
                for j in range(2 * KC):
                    pt = pctp.tile([128, 128], F32, tag="pc_tp")
                    nc.tensor.transpose(pt[:], g_sb[:, j * 128:(j + 1) * 128], ident[:])
                    nc.vector.tensor_copy(gT_r[:, j, :], pt[:])

                respb = pc.tile([128, H], F32, tag="respb")
                nc.vector.tensor_add(respb[:], res[:], bp2_big[:])

                ps2 = pcmm.tile([128, 512], F32, tag="pc_mm")
                for j in range(2 * KC):
                    w_r = wp2a_r if j < KC else wp2b_r
                    nc.tensor.matmul(ps2[:], gT_r[:, j, :], w_r[:, j % KC, :],
                                     start=(j == 0), stop=(j == 2 * KC - 1))
                t2 = pc.tile([128, H], F32, tag="t2")
                nc.vector.tensor_add(t2[:], ps2[:], respb[:])

                out_sb = pc.tile([128, H], F32, tag="osb")
                layernorm(out_sb, t2, ong_big, onb_big, "ln2")
                nc.sync.dma_start(out[:, :], out_sb[:])

    nc.compile()
    return nc


_CACHE = {}


def _get_nc():
    if "nc" not in _CACHE:
        _CACHE["nc"] = build()
    return _CACHE["nc"]


WEIGHT_NAMES = [
    "Wq", "bq", "Wk", "bk", "Wv", "bv", "Wo", "bo",
    "We1", "be1", "We2", "be2", "We3", "be3",
    "Wp1", "bp1", "Wp2", "bp2",
    "sa_g", "sa_b", "on_g", "on_b", "alpha", "beta",
]


def kernel(**inputs):
    from concourse.bass_utils import run_bass_kernel_spmd

    nc = _get_nc()
    x = np.ascontiguousarray(np.asarray(inputs["x"], dtype=np.float32))
    ea = np.ascontiguousarray(np.asarray(inputs["edge_attr"], dtype=np.float32))
    shared = {
        nm: np.ascontiguousarray(np.asarray(inputs[nm], dtype=np.float32))
        for nm in WEIGHT_NAMES
    }
    in_maps = []
    for c in range(BS):
        m = {"x": x[c], "edge_attr": ea[c]}
        m.update(shared)
        in_maps.append(m)
    res = run_bass_kernel_spmd(nc, in_maps, core_ids=list(range(BS)))
    return np.stack([res.results[c]["out"] for c in range(BS)], axis=0)
